# revision 3
# baseline (speedup 1.0000x reference)
"""BatchHardTripletLoss on 8 Trainium2 NeuronCores.

Strategy (data parallel over rows, all reductions in squared-distance space):
  - Sort rows by label (host). Each core owns a 1024-row block of the sorted
    order. Columns (candidate set = all 8192 rows) are rotated per-core so the
    core's own rows sit at fixed local columns [W/2, W/2+1024). Hence every
    row-tile's same-class columns fall inside a *fixed* local window
    [128*lt + W/2, 128*lt + W/2 + W), making one SPMD program valid for all
    cores (per-core variation lives entirely in the input data).
  - TensorE: psum[i, j] = sq[j] - 2*x_i.x_j via an f32r matmul plus a K=1
    ones-matmul accumulating sq[j] (broadcast along partitions).
  - VectorE: hardest-neg = min-reduce over all columns; same-class columns are
    confined to the window, where host-built +/-1e30 label masks are added
    (tensor_tensor) before min/max reduces. hardest-pos = masked max over the
    window (self included; singleton classes fixed up on host via counts).
  - Host: + sq[i], clamp, sqrt (eps rule), validity from label counts,
    margin + masked mean. sqrt is monotone, so reducing in squared space is
    exact.
"""

import numpy as np

N = 8192
D = 128
NUM_CLASSES = 512
MARGIN = 0.3
NCORES = 8
ROWS_PER_CORE = N // NCORES          # 1024
RT_PER_CORE = ROWS_PER_CORE // 128   # 8 row-tiles of 128 rows
GROUP = 2048                         # psum group (4 banks)
NGROUP = N // GROUP                  # 4
MMN = 512                            # matmul moving size / psum bank
BIG = 1.0e30

_PROGRAM_CACHE = {}


def _build_program(W):
    """Build + compile the SPMD program for band-window width W."""
    import concourse.mybir as mybir
    from concourse import bacc
    from concourse.tile import TileContext

    F32 = mybir.dt.float32
    F32R = mybir.dt.float32r

    nc = bacc.Bacc("TRN2", target_bir_lowering=False, debug=False,
                   num_devices=NCORES)

    featsT_d = nc.dram_tensor("featsT", [D, N], F32R, kind="ExternalInput")
    sqrow_d = nc.dram_tensor("sqrow", [1, N], F32R, kind="ExternalInput")
    ones_d = nc.dram_tensor("ones", [1, 128], F32R, kind="ExternalInput")
    rows2_d = nc.dram_tensor("rows2", [D, ROWS_PER_CORE], F32R,
                             kind="ExternalInput")
    negmask_d = nc.dram_tensor("negmask", [D, RT_PER_CORE * W], F32,
                               kind="ExternalInput")
    posmask_d = nc.dram_tensor("posmask", [D, RT_PER_CORE * W], F32,
                               kind="ExternalInput")
    neg_out_d = nc.dram_tensor("neg_out", [D, RT_PER_CORE], F32,
                               kind="ExternalOutput")
    pos_out_d = nc.dram_tensor("pos_out", [D, RT_PER_CORE], F32,
                               kind="ExternalOutput")

    with TileContext(nc) as tc:
        with (
            tc.tile_pool(name="big", bufs=1) as big,
            tc.tile_pool(name="psum", bufs=2, space="PSUM") as psum_pool,
            tc.tile_pool(name="scr", bufs=4) as scr,
            tc.tile_pool(name="small", bufs=1) as small,
        ):
            featsT = big.tile([D, N], F32R, tag="featsT")
            # chunked load so early matmuls start before the full 4MB lands
            for ch in range(8):
                nc.sync.dma_start(
                    featsT[:, ch * 1024:(ch + 1) * 1024],
                    featsT_d[:, ch * 1024:(ch + 1) * 1024],
                )
            sqrow = small.tile([1, N], F32R, tag="sqrow")
            nc.sync.dma_start(sqrow[:, :], sqrow_d[:, :])
            ones = small.tile([1, 128], F32R, tag="ones")
            nc.sync.dma_start(ones[:, :], ones_d[:, :])
            rows2 = big.tile([D, ROWS_PER_CORE], F32R, tag="rows2")
            nc.sync.dma_start(rows2[:, :], rows2_d[:, :])
            negmask = big.tile([D, RT_PER_CORE * W], F32, tag="negmask")
            nc.sync.dma_start(negmask[:, :], negmask_d[:, :])
            posmask = big.tile([D, RT_PER_CORE * W], F32, tag="posmask")
            nc.sync.dma_start(posmask[:, :], posmask_d[:, :])

            neg_sb = small.tile([D, RT_PER_CORE], F32, tag="neg_sb")
            pos_sb = small.tile([D, RT_PER_CORE], F32, tag="pos_sb")

            for lt in range(RT_PER_CORE):
                lhsT = rows2[:, 128 * lt:128 * (lt + 1)]
                partials = scr.tile([D, 8], F32, tag="partials")
                w0 = 128 * lt + 64  # window start (local cols, group 0)
                for g in range(NGROUP):
                    ps = psum_pool.tile([D, GROUP], F32, tag="ps")
                    for k in range(GROUP // MMN):
                        c0 = g * GROUP + k * MMN
                        nc.tensor.matmul(
                            ps[:, k * MMN:(k + 1) * MMN], lhsT,
                            featsT[:, c0:c0 + MMN], start=True, stop=False)
                        nc.tensor.matmul(
                            ps[:, k * MMN:(k + 1) * MMN], ones,
                            sqrow[:, c0:c0 + MMN], start=False, stop=True)
                    if g == 0:
                        # plain parts around the band window
                        nc.vector.tensor_reduce(
                            partials[:, 0:1], ps[:, 0:w0],
                            axis=mybir.AxisListType.X, op=mybir.AluOpType.min)
                        nc.vector.tensor_reduce(
                            partials[:, 1:2], ps[:, w0 + W:GROUP],
                            axis=mybir.AxisListType.X, op=mybir.AluOpType.min)
                        # band: masked pos (max) and neg (min)
                        scrP = scr.tile([D, W], F32, tag="scrP")
                        nc.vector.tensor_tensor(
                            out=scrP[:, :], in0=ps[:, w0:w0 + W],
                            in1=posmask[:, lt * W:(lt + 1) * W],
                            op=mybir.AluOpType.add)
                        nc.vector.tensor_reduce(
                            pos_sb[:, lt:lt + 1], scrP[:, :],
                            axis=mybir.AxisListType.X, op=mybir.AluOpType.max)
                        scrN = scr.tile([D, W], F32, tag="scrN")
                        nc.vector.tensor_tensor(
                            out=scrN[:, :], in0=ps[:, w0:w0 + W],
                            in1=negmask[:, lt * W:(lt + 1) * W],
                            op=mybir.AluOpType.add)
                        nc.vector.tensor_reduce(
                            partials[:, 2:3], scrN[:, :],
                            axis=mybir.AxisListType.X, op=mybir.AluOpType.min)
                    else:
                        nc.vector.tensor_reduce(
                            partials[:, 2 + g:3 + g], ps[:, :],
                            axis=mybir.AxisListType.X, op=mybir.AluOpType.min)
                nc.vector.tensor_reduce(
                    neg_sb[:, lt:lt + 1], partials[:, 0:6],
                    axis=mybir.AxisListType.X, op=mybir.AluOpType.min)

            nc.sync.dma_start(neg_out_d[:, :], neg_sb[:, :])
            nc.sync.dma_start(pos_out_d[:, :], pos_sb[:, :])

    nc.compile()
    return nc


def kernel(feats, labels):
    from concourse.bass_utils import run_bass_kernel_spmd

    feats = np.asarray(feats, dtype=np.float32)
    labels_np = np.asarray(labels).astype(np.int64)

    order = np.argsort(labels_np, kind="stable")
    feats_s = feats[order]                      # [N, D] sorted by label
    labels_s = labels_np[order]

    counts = np.bincount(labels_s, minlength=max(int(labels_s.max()) + 1, 1))
    mc = int(counts.max())
    if mc <= 65:
        W = 256
    elif mc <= 129:
        W = 384
    elif mc <= 193:
        W = 512
    else:
        raise ValueError(f"class of size {mc} exceeds supported band window")

    if W not in _PROGRAM_CACHE:
        _PROGRAM_CACHE[W] = _build_program(W)
    nc = _PROGRAM_CACHE[W]

    sq = np.einsum("nd,nd->n", feats_s.astype(np.float64),
                   feats_s.astype(np.float64)).astype(np.float32)
    ones_np = np.ones((1, 128), dtype=np.float32)

    in_maps = []
    for c in range(NCORES):
        rot = (ROWS_PER_CORE * c - W // 2) % N
        loc = (rot + np.arange(N)) % N          # local col -> global sorted row
        featsT_c = np.ascontiguousarray(feats_s[loc].T)
        rows2_c = np.ascontiguousarray(
            (-2.0 * feats_s[ROWS_PER_CORE * c:ROWS_PER_CORE * (c + 1)]).T)
        sqrow_c = np.ascontiguousarray(sq[loc][None, :])
        negmask_c = np.zeros((D, RT_PER_CORE * W), dtype=np.float32)
        posmask_c = np.zeros((D, RT_PER_CORE * W), dtype=np.float32)
        for lt in range(RT_PER_CORE):
            rows_lab = labels_s[ROWS_PER_CORE * c + 128 * lt:
                                ROWS_PER_CORE * c + 128 * (lt + 1)]
            w0 = 128 * lt + 64
            win_lab = labels_s[loc[w0:w0 + W]]
            same = rows_lab[:, None] == win_lab[None, :]
            negmask_c[:, lt * W:(lt + 1) * W] = np.where(same, BIG, 0.0)
            posmask_c[:, lt * W:(lt + 1) * W] = np.where(same, 0.0, -BIG)
        in_maps.append({
            "featsT": featsT_c,
            "sqrow": sqrow_c,
            "ones": ones_np,
            "rows2": rows2_c,
            "negmask": negmask_c,
            "posmask": posmask_c,
        })

    res = run_bass_kernel_spmd(nc, in_maps, core_ids=list(range(NCORES)))

    neg_raw = np.empty(N, dtype=np.float32)
    pos_raw = np.empty(N, dtype=np.float32)
    for c in range(NCORES):
        base = ROWS_PER_CORE * c
        neg_raw[base:base + ROWS_PER_CORE] = \
            res.results[c]["neg_out"].T.reshape(ROWS_PER_CORE)
        pos_raw[base:base + ROWS_PER_CORE] = \
            res.results[c]["pos_out"].T.reshape(ROWS_PER_CORE)

    # host epilogue (squared space -> distances -> loss), all fp32 like the ref
    hn_sq = np.maximum(neg_raw + sq, 0.0).astype(np.float32)
    hp_sq = np.maximum(pos_raw + sq, 0.0).astype(np.float32)
    eps = np.float32(1e-12)
    hn = np.where(hn_sq > eps, np.sqrt(hn_sq), np.float32(0.0))
    hp = np.where(hp_sq > eps, np.sqrt(hp_sq), np.float32(0.0))

    cnt_per_row = counts[labels_s]
    valid = (cnt_per_row >= 2) & (cnt_per_row < N)
    diff = np.where(valid, hp - hn, np.float32(0.0))
    per_row = np.maximum(diff + np.float32(MARGIN), np.float32(0.0))
    per_row = np.where(valid, per_row, np.float32(0.0)).astype(np.float32)
    cnt = np.float32(valid.sum())
    if cnt > 0:
        loss = np.float32(per_row.sum(dtype=np.float32) / max(cnt, np.float32(1.0)))
    else:
        loss = np.float32(0.0)
    return np.float32(loss)


# revision 4
# speedup vs baseline: 1.6392x; 1.6392x over previous
"""BatchHardTripletLoss on 8 Trainium2 NeuronCores.

Strategy (data parallel over rows, all reductions in squared-distance space):
  - Sort rows by label (host). Each core owns a 1024-row block of the sorted
    order. Columns (candidate set = all 8192 rows) are rotated per-core so the
    core's own rows sit at fixed local columns [W/2, W/2+1024). Hence every
    row-tile's same-class columns fall inside a *fixed* local window
    [128*lt + W/2, 128*lt + W/2 + W), making one SPMD program valid for all
    cores (per-core variation lives entirely in the input data).
  - TensorE: psum[i, j] = sq[j] - 2*x_i.x_j via an f32r matmul plus a K=1
    ones-matmul accumulating sq[j] (broadcast along partitions).
  - VectorE: hardest-neg = min-reduce over all columns; same-class columns are
    confined to the window, where host-built +/-1e30 label masks are added
    (tensor_tensor) before min/max reduces. hardest-pos = masked max over the
    window (self included; singleton classes fixed up on host via counts).
  - Host: + sq[i], clamp, sqrt (eps rule), validity from label counts,
    margin + masked mean. sqrt is monotone, so reducing in squared space is
    exact.
"""

import numpy as np

N = 8192
D = 128
NUM_CLASSES = 512
MARGIN = 0.3
NCORES = 8
ROWS_PER_CORE = N // NCORES          # 1024
RT_PER_CORE = ROWS_PER_CORE // 128   # 8 row-tiles of 128 rows
GROUP = 2048                         # psum group (4 banks)
NGROUP = N // GROUP                  # 4
MMN = 512                            # matmul moving size / psum bank
BIG = 1.0e30

_PROGRAM_CACHE = {}


def _build_program(W):
    """Build + compile the SPMD program for band-window width W."""
    import concourse.mybir as mybir
    from concourse import bacc
    from concourse.tile import TileContext

    F32 = mybir.dt.float32
    F32R = mybir.dt.float32r

    nc = bacc.Bacc("TRN2", target_bir_lowering=False, debug=False,
                   num_devices=NCORES)

    featsT_d = nc.dram_tensor("featsT", [D, N], F32R, kind="ExternalInput")
    sqb_d = nc.dram_tensor("sqb", [D, N], F32R, kind="ExternalInput")
    ones_d = nc.dram_tensor("ones", [D, 128], F32R, kind="ExternalInput")
    rows2_d = nc.dram_tensor("rows2", [D, ROWS_PER_CORE], F32R,
                             kind="ExternalInput")
    negmask_d = nc.dram_tensor("negmask", [D, RT_PER_CORE * W], F32,
                               kind="ExternalInput")
    posmask_d = nc.dram_tensor("posmask", [D, RT_PER_CORE * W], F32,
                               kind="ExternalInput")
    neg_out_d = nc.dram_tensor("neg_out", [D, RT_PER_CORE], F32,
                               kind="ExternalOutput")
    pos_out_d = nc.dram_tensor("pos_out", [D, RT_PER_CORE], F32,
                               kind="ExternalOutput")

    with TileContext(nc) as tc:
        with (
            tc.tile_pool(name="big", bufs=1) as big,
            tc.tile_pool(name="psum", bufs=2, space="PSUM") as psum_pool,
            tc.tile_pool(name="scr", bufs=4) as scr,
            tc.tile_pool(name="small", bufs=1) as small,
        ):
            featsT = big.tile([D, N], F32R, tag="featsT")
            # chunked load so early matmuls start before the full 4MB lands
            for ch in range(8):
                nc.sync.dma_start(
                    featsT[:, ch * 1024:(ch + 1) * 1024],
                    featsT_d[:, ch * 1024:(ch + 1) * 1024],
                )
            sqb = big.tile([D, N], F32R, tag="sqb")
            for ch in range(8):
                nc.sync.dma_start(
                    sqb[:, ch * 1024:(ch + 1) * 1024],
                    sqb_d[:, ch * 1024:(ch + 1) * 1024],
                )
            ones = small.tile([D, 128], F32R, tag="ones")
            nc.sync.dma_start(ones[:, :], ones_d[:, :])
            rows2 = big.tile([D, ROWS_PER_CORE], F32R, tag="rows2")
            nc.sync.dma_start(rows2[:, :], rows2_d[:, :])
            negmask = big.tile([D, RT_PER_CORE * W], F32, tag="negmask")
            nc.sync.dma_start(negmask[:, :], negmask_d[:, :])
            posmask = big.tile([D, RT_PER_CORE * W], F32, tag="posmask")
            nc.sync.dma_start(posmask[:, :], posmask_d[:, :])

            neg_sb = small.tile([D, RT_PER_CORE], F32, tag="neg_sb")
            pos_sb = small.tile([D, RT_PER_CORE], F32, tag="pos_sb")

            for lt in range(RT_PER_CORE):
                lhsT = rows2[:, 128 * lt:128 * (lt + 1)]
                partials = scr.tile([D, 8], F32, tag="partials")
                w0 = 128 * lt + 64  # window start (local cols, group 0)
                for g in range(NGROUP):
                    ps = psum_pool.tile([D, GROUP], F32, tag="ps")
                    for k in range(GROUP // MMN):
                        c0 = g * GROUP + k * MMN
                        nc.tensor.matmul(
                            ps[:, k * MMN:(k + 1) * MMN], lhsT,
                            featsT[:, c0:c0 + MMN], start=True, stop=False)
                        nc.tensor.matmul(
                            ps[:, k * MMN:(k + 1) * MMN], ones,
                            sqb[:, c0:c0 + MMN], start=False, stop=True)
                    if g == 0:
                        # plain parts around the band window
                        nc.vector.tensor_reduce(
                            partials[:, 0:1], ps[:, 0:w0],
                            axis=mybir.AxisListType.X, op=mybir.AluOpType.min)
                        nc.vector.tensor_reduce(
                            partials[:, 1:2], ps[:, w0 + W:GROUP],
                            axis=mybir.AxisListType.X, op=mybir.AluOpType.min)
                        # band: masked pos (max) and neg (min)
                        scrP = scr.tile([D, W], F32, tag="scrP")
                        nc.vector.tensor_tensor(
                            out=scrP[:, :], in0=ps[:, w0:w0 + W],
                            in1=posmask[:, lt * W:(lt + 1) * W],
                            op=mybir.AluOpType.add)
                        nc.vector.tensor_reduce(
                            pos_sb[:, lt:lt + 1], scrP[:, :],
                            axis=mybir.AxisListType.X, op=mybir.AluOpType.max)
                        scrN = scr.tile([D, W], F32, tag="scrN")
                        nc.vector.tensor_tensor(
                            out=scrN[:, :], in0=ps[:, w0:w0 + W],
                            in1=negmask[:, lt * W:(lt + 1) * W],
                            op=mybir.AluOpType.add)
                        nc.vector.tensor_reduce(
                            partials[:, 2:3], scrN[:, :],
                            axis=mybir.AxisListType.X, op=mybir.AluOpType.min)
                    else:
                        nc.vector.tensor_reduce(
                            partials[:, 2 + g:3 + g], ps[:, :],
                            axis=mybir.AxisListType.X, op=mybir.AluOpType.min)
                nc.vector.tensor_reduce(
                    neg_sb[:, lt:lt + 1], partials[:, 0:6],
                    axis=mybir.AxisListType.X, op=mybir.AluOpType.min)

            nc.sync.dma_start(neg_out_d[:, :], neg_sb[:, :])
            nc.sync.dma_start(pos_out_d[:, :], pos_sb[:, :])

    nc.compile()
    return nc


def kernel(feats, labels):
    from concourse.bass_utils import run_bass_kernel_spmd

    feats = np.asarray(feats, dtype=np.float32)
    labels_np = np.asarray(labels).astype(np.int64)

    order = np.argsort(labels_np, kind="stable")
    feats_s = feats[order]                      # [N, D] sorted by label
    labels_s = labels_np[order]

    counts = np.bincount(labels_s, minlength=max(int(labels_s.max()) + 1, 1))
    mc = int(counts.max())
    if mc <= 65:
        W = 256
    elif mc <= 129:
        W = 384
    elif mc <= 193:
        W = 512
    else:
        raise ValueError(f"class of size {mc} exceeds supported band window")

    if W not in _PROGRAM_CACHE:
        _PROGRAM_CACHE[W] = _build_program(W)
    nc = _PROGRAM_CACHE[W]

    sq = np.einsum("nd,nd->n", feats_s.astype(np.float64),
                   feats_s.astype(np.float64)).astype(np.float32)
    ones_np = np.ones((D, 128), dtype=np.float32)

    in_maps = []
    for c in range(NCORES):
        rot = (ROWS_PER_CORE * c - W // 2) % N
        loc = (rot + np.arange(N)) % N          # local col -> global sorted row
        featsT_c = np.ascontiguousarray(feats_s[loc].T)
        rows2_c = np.ascontiguousarray(
            (-2.0 * feats_s[ROWS_PER_CORE * c:ROWS_PER_CORE * (c + 1)]).T)
        sqb_c = np.ascontiguousarray(
            np.broadcast_to((sq[loc] / 128.0)[None, :], (D, N)))
        negmask_c = np.zeros((D, RT_PER_CORE * W), dtype=np.float32)
        posmask_c = np.zeros((D, RT_PER_CORE * W), dtype=np.float32)
        for lt in range(RT_PER_CORE):
            rows_lab = labels_s[ROWS_PER_CORE * c + 128 * lt:
                                ROWS_PER_CORE * c + 128 * (lt + 1)]
            w0 = 128 * lt + 64
            win_lab = labels_s[loc[w0:w0 + W]]
            same = rows_lab[:, None] == win_lab[None, :]
            negmask_c[:, lt * W:(lt + 1) * W] = np.where(same, BIG, 0.0)
            posmask_c[:, lt * W:(lt + 1) * W] = np.where(same, 0.0, -BIG)
        in_maps.append({
            "featsT": featsT_c,
            "sqb": sqb_c,
            "ones": ones_np,
            "rows2": rows2_c,
            "negmask": negmask_c,
            "posmask": posmask_c,
        })

    res = run_bass_kernel_spmd(nc, in_maps, core_ids=list(range(NCORES)))

    neg_raw = np.empty(N, dtype=np.float32)
    pos_raw = np.empty(N, dtype=np.float32)
    for c in range(NCORES):
        base = ROWS_PER_CORE * c
        neg_raw[base:base + ROWS_PER_CORE] = \
            res.results[c]["neg_out"].T.reshape(ROWS_PER_CORE)
        pos_raw[base:base + ROWS_PER_CORE] = \
            res.results[c]["pos_out"].T.reshape(ROWS_PER_CORE)

    # host epilogue (squared space -> distances -> loss), all fp32 like the ref
    hn_sq = np.maximum(neg_raw + sq, 0.0).astype(np.float32)
    hp_sq = np.maximum(pos_raw + sq, 0.0).astype(np.float32)
    eps = np.float32(1e-12)
    hn = np.where(hn_sq > eps, np.sqrt(hn_sq), np.float32(0.0))
    hp = np.where(hp_sq > eps, np.sqrt(hp_sq), np.float32(0.0))

    cnt_per_row = counts[labels_s]
    valid = (cnt_per_row >= 2) & (cnt_per_row < N)
    diff = np.where(valid, hp - hn, np.float32(0.0))
    per_row = np.maximum(diff + np.float32(MARGIN), np.float32(0.0))
    per_row = np.where(valid, per_row, np.float32(0.0)).astype(np.float32)
    cnt = np.float32(valid.sum())
    if cnt > 0:
        loss = np.float32(per_row.sum(dtype=np.float32) / max(cnt, np.float32(1.0)))
    else:
        loss = np.float32(0.0)
    return np.float32(loss)


# revision 6
# speedup vs baseline: 1.8574x; 1.1331x over previous
"""BatchHardTripletLoss on 8 Trainium2 NeuronCores.

Strategy (data parallel over rows, all reductions in squared-distance space):
  - Sort rows by label (host). Each core owns a 1024-row block of the sorted
    order. Columns (candidate set = all 8192 rows) are rotated per-core so the
    core's own rows sit at fixed local columns [W/2, W/2+1024). Hence every
    row-tile's same-class columns fall inside a *fixed* local window
    [128*lt + W/2, 128*lt + W/2 + W), making one SPMD program valid for all
    cores (per-core variation lives entirely in the input data).
  - TensorE: psum[i, j] = sq[j] - 2*x_i.x_j via an f32r matmul plus a K=1
    ones-matmul accumulating sq[j] (broadcast along partitions).
  - VectorE: hardest-neg = min-reduce over all columns; same-class columns are
    confined to the window, where host-built +/-1e30 label masks are added
    (tensor_tensor) before min/max reduces. hardest-pos = masked max over the
    window (self included; singleton classes fixed up on host via counts).
  - Host: + sq[i], clamp, sqrt (eps rule), validity from label counts,
    margin + masked mean. sqrt is monotone, so reducing in squared space is
    exact.
"""

import numpy as np

N = 8192
D = 128
NUM_CLASSES = 512
MARGIN = 0.3
NCORES = 8
ROWS_PER_CORE = N // NCORES          # 1024
RT_PER_CORE = ROWS_PER_CORE // 128   # 8 row-tiles of 128 rows
GROUP = 2048                         # psum group (4 banks)
NGROUP = N // GROUP                  # 4
MMN = 512                            # matmul moving size / psum bank
BIG = 1.0e30

_PROGRAM_CACHE = {}


def _build_program(W):
    """Build + compile the SPMD program for band-window width W."""
    import concourse.mybir as mybir
    from concourse import bacc
    from concourse.tile import TileContext

    F32 = mybir.dt.float32
    F32R = mybir.dt.float32r

    nc = bacc.Bacc("TRN2", target_bir_lowering=False, debug=False,
                   num_devices=NCORES)

    featsT_d = nc.dram_tensor("featsT", [D, N], F32R, kind="ExternalInput")
    sqb_d = nc.dram_tensor("sqb", [D, N], F32R, kind="ExternalInput")
    ones_d = nc.dram_tensor("ones", [D, 128], F32R, kind="ExternalInput")
    rows2_d = nc.dram_tensor("rows2", [D, ROWS_PER_CORE], F32R,
                             kind="ExternalInput")
    negmask_d = nc.dram_tensor("negmask", [D, RT_PER_CORE * W], F32,
                               kind="ExternalInput")
    posmask_d = nc.dram_tensor("posmask", [D, RT_PER_CORE * W], F32,
                               kind="ExternalInput")
    neg_out_d = nc.dram_tensor("neg_out", [D, RT_PER_CORE], F32,
                               kind="ExternalOutput")
    pos_out_d = nc.dram_tensor("pos_out", [D, RT_PER_CORE], F32,
                               kind="ExternalOutput")

    with TileContext(nc) as tc:
        with (
            tc.tile_pool(name="big", bufs=1) as big,
            tc.tile_pool(name="psum", bufs=2, space="PSUM") as psum_pool,
            tc.tile_pool(name="scr", bufs=4) as scr,
            tc.tile_pool(name="small", bufs=1) as small,
        ):
            featsT = big.tile([D, N], F32R, tag="featsT")
            # chunked load so early matmuls start before the full 4MB lands
            for ch in range(8):
                nc.sync.dma_start(
                    featsT[:, ch * 1024:(ch + 1) * 1024],
                    featsT_d[:, ch * 1024:(ch + 1) * 1024],
                )
            sqb = big.tile([D, N], F32R, tag="sqb")
            for ch in range(8):
                nc.sync.dma_start(
                    sqb[:, ch * 1024:(ch + 1) * 1024],
                    sqb_d[:, ch * 1024:(ch + 1) * 1024],
                )
            ones = small.tile([D, 128], F32R, tag="ones")
            nc.sync.dma_start(ones[:, :], ones_d[:, :])
            rows2 = big.tile([D, ROWS_PER_CORE], F32R, tag="rows2")
            nc.sync.dma_start(rows2[:, :], rows2_d[:, :])
            negmask = big.tile([D, RT_PER_CORE * W], F32, tag="negmask")
            nc.sync.dma_start(negmask[:, :], negmask_d[:, :])
            posmask = big.tile([D, RT_PER_CORE * W], F32, tag="posmask")
            nc.sync.dma_start(posmask[:, :], posmask_d[:, :])

            neg_sb = small.tile([D, RT_PER_CORE], F32, tag="neg_sb")
            pos_sb = small.tile([D, RT_PER_CORE], F32, tag="pos_sb")

            for lt in range(RT_PER_CORE):
                lhsT = rows2[:, 128 * lt:128 * (lt + 1)]
                partials = scr.tile([D, 8], F32, tag="partials")
                w0 = 128 * lt + 64  # window start (local cols, group 0)
                for g in range(NGROUP):
                    ps = psum_pool.tile([D, GROUP], F32, tag="ps")
                    for k in range(GROUP // MMN):
                        c0 = g * GROUP + k * MMN
                        nc.tensor.matmul(
                            ps[:, k * MMN:(k + 1) * MMN], lhsT,
                            featsT[:, c0:c0 + MMN], start=True, stop=False)
                        nc.tensor.matmul(
                            ps[:, k * MMN:(k + 1) * MMN], ones,
                            sqb[:, c0:c0 + MMN], start=False, stop=True)
                    if g == 0:
                        # band: masked pos (max) into pos_sb via scratch
                        scrP = scr.tile([D, W], F32, tag="scrP")
                        nc.vector.tensor_tensor(
                            out=scrP[:, :], in0=ps[:, w0:w0 + W],
                            in1=posmask[:, lt * W:(lt + 1) * W],
                            op=mybir.AluOpType.add)
                        nc.vector.tensor_reduce(
                            pos_sb[:, lt:lt + 1], scrP[:, :],
                            axis=mybir.AxisListType.X, op=mybir.AluOpType.max)
                        # band neg: mask same-class in place, then one reduce
                        nc.vector.tensor_tensor(
                            out=ps[:, w0:w0 + W], in0=ps[:, w0:w0 + W],
                            in1=negmask[:, lt * W:(lt + 1) * W],
                            op=mybir.AluOpType.add)
                        nc.vector.tensor_reduce(
                            partials[:, 2:3], ps[:, :],
                            axis=mybir.AxisListType.X, op=mybir.AluOpType.min)
                    else:
                        nc.vector.tensor_reduce(
                            partials[:, 2 + g:3 + g], ps[:, :],
                            axis=mybir.AxisListType.X, op=mybir.AluOpType.min)
                nc.vector.tensor_reduce(
                    neg_sb[:, lt:lt + 1], partials[:, 2:6],
                    axis=mybir.AxisListType.X, op=mybir.AluOpType.min)

            nc.sync.dma_start(neg_out_d[:, :], neg_sb[:, :])
            nc.sync.dma_start(pos_out_d[:, :], pos_sb[:, :])

    nc.compile()
    return nc


def kernel(feats, labels):
    from concourse.bass_utils import run_bass_kernel_spmd

    feats = np.asarray(feats, dtype=np.float32)
    labels_np = np.asarray(labels).astype(np.int64)

    order = np.argsort(labels_np, kind="stable")
    feats_s = feats[order]                      # [N, D] sorted by label
    labels_s = labels_np[order]

    counts = np.bincount(labels_s, minlength=max(int(labels_s.max()) + 1, 1))
    mc = int(counts.max())
    if mc <= 65:
        W = 256
    elif mc <= 129:
        W = 384
    elif mc <= 193:
        W = 512
    else:
        raise ValueError(f"class of size {mc} exceeds supported band window")

    if W not in _PROGRAM_CACHE:
        _PROGRAM_CACHE[W] = _build_program(W)
    nc = _PROGRAM_CACHE[W]

    sq = np.einsum("nd,nd->n", feats_s.astype(np.float64),
                   feats_s.astype(np.float64)).astype(np.float32)
    ones_np = np.ones((D, 128), dtype=np.float32)

    in_maps = []
    for c in range(NCORES):
        rot = (ROWS_PER_CORE * c - W // 2) % N
        loc = (rot + np.arange(N)) % N          # local col -> global sorted row
        featsT_c = np.ascontiguousarray(feats_s[loc].T)
        rows2_c = np.ascontiguousarray(
            (-2.0 * feats_s[ROWS_PER_CORE * c:ROWS_PER_CORE * (c + 1)]).T)
        sqb_c = np.ascontiguousarray(
            np.broadcast_to((sq[loc] / 128.0)[None, :], (D, N)))
        negmask_c = np.zeros((D, RT_PER_CORE * W), dtype=np.float32)
        posmask_c = np.zeros((D, RT_PER_CORE * W), dtype=np.float32)
        for lt in range(RT_PER_CORE):
            rows_lab = labels_s[ROWS_PER_CORE * c + 128 * lt:
                                ROWS_PER_CORE * c + 128 * (lt + 1)]
            w0 = 128 * lt + 64
            win_lab = labels_s[loc[w0:w0 + W]]
            same = rows_lab[:, None] == win_lab[None, :]
            negmask_c[:, lt * W:(lt + 1) * W] = np.where(same, BIG, 0.0)
            posmask_c[:, lt * W:(lt + 1) * W] = np.where(same, 0.0, -BIG)
        in_maps.append({
            "featsT": featsT_c,
            "sqb": sqb_c,
            "ones": ones_np,
            "rows2": rows2_c,
            "negmask": negmask_c,
            "posmask": posmask_c,
        })

    res = run_bass_kernel_spmd(nc, in_maps, core_ids=list(range(NCORES)))

    neg_raw = np.empty(N, dtype=np.float32)
    pos_raw = np.empty(N, dtype=np.float32)
    for c in range(NCORES):
        base = ROWS_PER_CORE * c
        neg_raw[base:base + ROWS_PER_CORE] = \
            res.results[c]["neg_out"].T.reshape(ROWS_PER_CORE)
        pos_raw[base:base + ROWS_PER_CORE] = \
            res.results[c]["pos_out"].T.reshape(ROWS_PER_CORE)

    # host epilogue (squared space -> distances -> loss), all fp32 like the ref
    hn_sq = np.maximum(neg_raw + sq, 0.0).astype(np.float32)
    hp_sq = np.maximum(pos_raw + sq, 0.0).astype(np.float32)
    eps = np.float32(1e-12)
    hn = np.where(hn_sq > eps, np.sqrt(hn_sq), np.float32(0.0))
    hp = np.where(hp_sq > eps, np.sqrt(hp_sq), np.float32(0.0))

    cnt_per_row = counts[labels_s]
    valid = (cnt_per_row >= 2) & (cnt_per_row < N)
    diff = np.where(valid, hp - hn, np.float32(0.0))
    per_row = np.maximum(diff + np.float32(MARGIN), np.float32(0.0))
    per_row = np.where(valid, per_row, np.float32(0.0)).astype(np.float32)
    cnt = np.float32(valid.sum())
    if cnt > 0:
        loss = np.float32(per_row.sum(dtype=np.float32) / max(cnt, np.float32(1.0)))
    else:
        loss = np.float32(0.0)
    return np.float32(loss)
